# revision 13
# baseline (speedup 1.0000x reference)
"""CoAttention GNN message passing kernel for 8 Trainium2 NeuronCores.

Strategy:
  - Edges are binned on the host into chunks such that within each chunk all
    seg_i1 are distinct and all seg_i2 are distinct (required because the HW
    scatter-add CCE path races on duplicate rows within one call). Chunks are
    padded to a fixed size with dummy edges pointing at a dummy table region
    whose K rows are +/-10 so exp((k1.k2)/8) underflows to exactly 0.
  - Each core processes NCHUNK chunks: dma_gather of [K|V] rows by seg ids,
    computes t = <K1,K2>, e = exp(t/8), then dma_scatter_adds [e*V | e] rows
    into per-core partial tables U1 (by seg_i1) and U2 (by seg_i2).
  - ReduceScatter sums the partial tables; each core normalizes its owned
    node slice, applies the output projection + leaky relu, and publishes
    r = 1/(s+eps) via a small AllGather.
  - A second edge pass gathers r rows to produce the per-edge softmax weights.
  - The segment-max subtraction of the reference cancels algebraically; the
    eps difference is bounded by ~1e-8 relative (max term dominates the sum).
"""
import sys

sys.path.insert(0, "/opt/trn_rl_repo")

import numpy as np

P = 128


def _bin_edges(i1, i2, cap):
    """Partition edge ids into bins with per-bin distinct i1 and distinct i2."""
    E = len(i1)
    remaining = np.arange(E, dtype=np.int64)
    bins = []
    placed = np.zeros(E, dtype=bool)
    while remaining.size:
        r1 = i1[remaining]
        _, fi = np.unique(r1, return_index=True)
        cand = remaining[fi]
        r2 = i2[cand]
        _, fi2 = np.unique(r2, return_index=True)
        sel = cand[fi2]
        if sel.size > cap:
            sel = sel[:cap]
        bins.append(sel)
        placed[sel] = True
        remaining = remaining[~placed[remaining]]
    return bins


def _wrap_idx(ids, cap):
    """int16 ids [cap] -> dma_gather/scatter SBUF layout [128, cap//16]."""
    w = ids.reshape(cap // 16, 16).T  # [16, cap//16]
    return np.tile(w, (8, 1)).astype(np.int16)


def _run_spmd(nc, in_maps, n_cores):
    """Execute the finalized Bass module on n_cores via PJRT (axon), keeping
    the jitted executable alive so an optional second call measures pure
    device execution time (COATT_TIME=1)."""
    import os
    import time

    import jax
    from jax.experimental.shard_map import shard_map
    from jax.sharding import Mesh, PartitionSpec

    from concourse import mybir
    from concourse.bass2jax import (_bass_exec_p, install_neuronx_cc_hook,
                                    partition_id_tensor)

    install_neuronx_cc_hook()
    if nc.dbg_addr is not None:
        in_maps = [
            {**m, nc.dbg_addr.name: np.zeros((1, 2), np.uint32)} for m in in_maps
        ]
    partition_name = nc.partition_id_tensor.name if nc.partition_id_tensor else None
    in_names, out_names, out_avals, zero_outs = [], [], [], []
    for alloc in nc.m.functions[0].allocations:
        if not isinstance(alloc, mybir.MemoryLocationSet):
            continue
        name = alloc.memorylocations[0].name
        if alloc.kind == "ExternalInput":
            if name != partition_name:
                in_names.append(name)
        elif alloc.kind == "ExternalOutput":
            out_names.append(name)
            out_avals.append(jax.core.ShapedArray(
                tuple(alloc.tensor_shape), mybir.dt.np(alloc.dtype)))
            zero_outs.append(np.zeros(
                tuple(alloc.tensor_shape), mybir.dt.np(alloc.dtype)))
    n_params = len(in_names)
    n_outs = len(out_names)
    in_names = in_names + out_names
    if partition_name is not None:
        in_names.append(partition_name)
    donate = tuple(range(n_params, n_params + n_outs))

    def _body(*args):
        operands = list(args)
        if partition_name is not None:
            operands.append(partition_id_tensor())
        return tuple(_bass_exec_p.bind(
            *operands, out_avals=tuple(out_avals), in_names=tuple(in_names),
            out_names=tuple(out_names), lowering_input_output_aliases=(),
            sim_require_finite=True, sim_require_nnan=True, nc=nc))

    devices = jax.devices()[:n_cores]
    mesh = Mesh(np.asarray(devices), ("core",))
    sharded = jax.jit(
        shard_map(_body, mesh=mesh,
                  in_specs=(PartitionSpec("core"),) * (n_params + n_outs),
                  out_specs=(PartitionSpec("core"),) * n_outs,
                  check_rep=False),
        donate_argnums=donate, keep_unused=True)
    per_core = [[np.asarray(m[name]) for name in in_names[:n_params]]
                for m in in_maps]
    concat_in = [np.concatenate([per_core[c][i] for c in range(n_cores)], axis=0)
                 for i in range(n_params)]
    concat_zeros = [np.zeros((n_cores * z.shape[0], *z.shape[1:]), z.dtype)
                    for z in zero_outs]
    out_arrs = jax.block_until_ready(sharded(*concat_in, *concat_zeros))
    results = [
        {name: np.asarray(out_arrs[i]).reshape(n_cores, *out_avals[i].shape)[c]
         for i, name in enumerate(out_names)}
        for c in range(n_cores)
    ]
    if os.environ.get("COATT_TIME"):
        sh_in = [a.sharding for a in out_arrs] and None
        in_dev = jax.block_until_ready(
            [jax.device_put(a, jax.sharding.NamedSharding(mesh, PartitionSpec("core")))
             for a in concat_in])
        best = None
        for _ in range(3):
            zs = jax.block_until_ready(
                [jax.device_put(np.zeros((n_cores * z.shape[0], *z.shape[1:]),
                                         z.dtype),
                                jax.sharding.NamedSharding(mesh, PartitionSpec("core")))
                 for z in zero_outs])
            t0 = time.perf_counter()
            o = jax.block_until_ready(sharded(*in_dev, *zs))
            dt = time.perf_counter() - t0
            best = dt if best is None else min(best, dt)
            del o
        print(f"HW exec time: {int(best * 1e9)} ns")
    return results


def kernel(node1, seg_i1, idx_j1, node2, seg_i2, idx_j2, Wk, Wv, Wo, bo, _debug=False):
    from concourse import bacc, mybir, tile
    from concourse.bass_utils import run_bass_kernel_spmd
    from concourse.masks import make_identity

    f32 = mybir.dt.float32
    i16 = mybir.dt.int16

    node1 = np.asarray(node1, np.float32)
    node2 = np.asarray(node2, np.float32)
    seg_i1 = np.asarray(seg_i1)
    seg_i2 = np.asarray(seg_i2)
    Wk = np.asarray(Wk, np.float32)
    Wv = np.asarray(Wv, np.float32)
    Wo = np.asarray(Wo, np.float32)
    bo = np.asarray(bo, np.float32)

    N1, D = node1.shape
    N2 = node2.shape[0]
    E = seg_i1.shape[0]
    assert N1 == N2 and D == 64
    NCORE = 8
    import os as _os
    CAP = int(_os.environ.get('COATT_CAP', 4096))  # edges per chunk
    C = CAP // P                    # gather rows per partition
    NODE_PAD = -(-N1 // 1024) * 1024
    TAB = NODE_PAD + CAP            # table rows incl dummy region
    OWN = NODE_PAD // NCORE         # nodes owned per core post-RS
    NBLK = NODE_PAD // P            # 128-node blocks for KV build
    OBLK = OWN // P                 # epilogue blocks per core
    HALF = NODE_PAD // 2
    A_HALF = HALF // P              # rows per partition in R build

    # ---------------- host: bin edges into race-free chunks ----------------
    bins = _bin_edges(seg_i1, seg_i2, CAP)
    NCHUNK = -(-len(bins) // NCORE)
    while len(bins) < NCHUNK * NCORE:
        bins.append(np.empty(0, dtype=np.int64))
    # balance: snake-deal by size
    order = np.argsort([-b.size for b in bins])
    core_bins = [[] for _ in range(NCORE)]
    for i, bi in enumerate(order):
        r = i // NCORE
        c = i % NCORE if r % 2 == 0 else NCORE - 1 - (i % NCORE)
        core_bins[c].append(bins[bi])

    idx1_np = np.zeros((NCORE, NCHUNK, P, CAP // 16), np.int16)
    idx2_np = np.zeros((NCORE, NCHUNK, P, CAP // 16), np.int16)
    members = []  # per (core, chunk): original edge ids (len<=CAP)
    for c in range(NCORE):
        mem_c = []
        for k in range(NCHUNK):
            b = core_bins[c][k]
            npad = CAP - b.size
            ids1 = np.concatenate([seg_i1[b], NODE_PAD + np.arange(npad)])
            ids2 = np.concatenate([seg_i2[b], NODE_PAD + np.arange(npad)])
            idx1_np[c, k] = _wrap_idx(ids1.astype(np.int16), CAP)
            idx2_np[c, k] = _wrap_idx(ids2.astype(np.int16), CAP)
            mem_c.append(b)
        members.append(mem_c)

    node1p = np.zeros((NODE_PAD, D), np.float32)
    node1p[:N1] = node1
    node2p = np.zeros((NODE_PAD, D), np.float32)
    node2p[:N2] = node2
    wkv = np.concatenate([Wk, Wv], axis=1).copy()  # [64, 128]
    bo2 = np.tile(bo.reshape(1, D), (P, 1)).copy()

    # ---------------- device program (SPMD, identical on all cores) --------
    nc = bacc.Bacc(None, target_bir_lowering=False, debug=True,
                   dynamic_dma_scratch_size=32768, num_swdge_queues=4)
    dp = nc.declare_dram_parameter
    n1_in = dp("node1p", [NODE_PAD, D], f32, isOutput=False)
    n2_in = dp("node2p", [NODE_PAD, D], f32, isOutput=False)
    wkv_in = dp("wkv", [D, 2 * D], f32, isOutput=False)
    wo_in = dp("wo", [D, D], f32, isOutput=False)
    bo_in = dp("bo", [P, D], f32, isOutput=False)
    idx1_in = dp("idx1", [NCHUNK, P, CAP // 16], i16, isOutput=False)
    idx2_in = dp("idx2", [NCHUNK, P, CAP // 16], i16, isOutput=False)
    out1_o = dp("out1s", [OWN, D], f32, isOutput=True)
    out2_o = dp("out2s", [OWN, D], f32, isOutput=True)
    a1_o = dp("a1s", [NCHUNK, P, C], f32, isOutput=True)
    a2_o = dp("a2s", [NCHUNK, P, C], f32, isOutput=True)

    KV1 = nc.dram_tensor("KV1", [TAB, 2 * D], f32)
    KV2 = nc.dram_tensor("KV2", [TAB, 2 * D], f32)
    U1 = nc.dram_tensor("U1", [TAB, 2 * D], f32)
    U2 = nc.dram_tensor("U2", [TAB, 2 * D], f32)
    Rb = nc.dram_tensor("Rb", [TAB, D], f32)
    e_buf = nc.dram_tensor("e_buf", [NCHUNK, P, C], f32)
    if _debug:
        dbg_outs = {
            "KV1": (KV1, [TAB, 2 * D]), "U1": (U1, [TAB, 2 * D]),
            "Rb": (Rb, [TAB, D]), "e_buf": (e_buf, [NCHUNK, P, C]),
        }
        dbg_decl = {k: dp(k + "_dbg", list(shp), f32, isOutput=True)
                    for k, (t_, shp) in dbg_outs.items()}
    rp_loc = nc.dram_tensor("rp_loc", [OWN, 2], f32)
    rp_full = nc.dram_tensor("rp_full", [NODE_PAD, 2], f32, addr_space="Shared")
    u1rs = nc.dram_tensor("u1rs", [OWN, 2 * D], f32)
    u2rs = nc.dram_tensor("u2rs", [OWN, 2 * D], f32)

    Exp = mybir.ActivationFunctionType.Exp
    AX = mybir.AxisListType.X
    add = mybir.AluOpType.add
    mult = mybir.AluOpType.mult
    amax = mybir.AluOpType.max
    groups = [list(range(NCORE))]

    with tile.TileContext(nc) as tc:
        with (
            tc.tile_pool(name="g", bufs=2 if CAP <= 4096 else 1) as gp,
            tc.tile_pool(name="m", bufs=2 if CAP <= 4096 else 1) as mp,
            tc.tile_pool(name="ix", bufs=2) as ixp,
            tc.tile_pool(name="sm", bufs=2) as smp,
            tc.tile_pool(name="ep", bufs=1) as epp,
            tc.tile_pool(name="ps", bufs=2, space="PSUM") as psp,
        ):
            # ---- phase 0: zero U tables, fill dummy regions, build KV ----
            zt = epp.tile([P, 4096], f32, tag="zero")
            nc.vector.memset(zt[:], 0.0)

            def zero_fill(region_ap, total):
                x = total // P
                v = region_ap.rearrange("(p x) -> p x", p=P)
                for off in range(0, x, 4096):
                    w = min(4096, x - off)
                    nc.sync.dma_start(out=v[:, off:off + w], in_=zt[:, 0:w])

            for t_ in (U1, U2):
                zero_fill(t_[:].rearrange("n d -> (n d)"), TAB * 2 * D)
            # Rb dummy region zeroed (pad gathers read it)
            zero_fill(Rb[NODE_PAD:TAB, :].rearrange("n d -> (n d)"), CAP * D)
            # KV dummy regions: K half +/-10, V half 0
            for t_, kval in ((KV1, 10.0), (KV2, -10.0)):
                pat = epp.tile([P, CAP // P, 2 * D], f32, tag="pat")
                nc.vector.memset(pat[:], 0.0)
                nc.vector.memset(pat[:, :, 0:D], kval)
                nc.sync.dma_start(
                    out=t_[NODE_PAD:TAB, :].rearrange("(p a) d -> p (a d)", p=P),
                    in_=pat[:].rearrange("p a d -> p (a d)"))

            ident = epp.tile([P, P], f32, tag="ident")
            make_identity(nc, ident[:])
            wkv_t = epp.tile([D, 2 * D], f32, tag="wkv")
            nc.sync.dma_start(out=wkv_t[:], in_=wkv_in[:])
            wo_t = epp.tile([D, D], f32, tag="wo")
            nc.sync.dma_start(out=wo_t[:], in_=wo_in[:])
            bo_t = epp.tile([P, D], f32, tag="bo")
            nc.sync.dma_start(out=bo_t[:], in_=bo_in[:])

            for b in range(NBLK):
                for src, dst in ((n1_in, KV1), (n2_in, KV2)):
                    nt = epp.tile([P, P], f32, tag="nblk")
                    nc.sync.dma_start(out=nt[:, 0:D], in_=src[b * P:(b + 1) * P, :])
                    ntp = psp.tile([P, P], f32, tag="ntp")
                    nc.tensor.transpose(out=ntp[:], in_=nt[:], identity=ident[:])
                    ntt = epp.tile([P, P], f32, tag="ntt")
                    nc.vector.tensor_copy(out=ntt[:], in_=ntp[:])
                    kvp = psp.tile([P, 2 * D], f32, tag="kvp")
                    nc.tensor.matmul(out=kvp[:], lhsT=ntt[0:D, :], rhs=wkv_t[:],
                                     start=True, stop=True)
                    kvs = epp.tile([P, 2 * D], f32, tag="kvs")
                    nc.vector.tensor_copy(out=kvs[:], in_=kvp[:])
                    nc.sync.dma_start(out=dst[b * P:(b + 1) * P, :], in_=kvs[:])

            # ---- main pass over edge chunks ----
            for k in range(NCHUNK):
                ix1 = ixp.tile([P, CAP // 16], i16, tag="ix1")
                nc.sync.dma_start(out=ix1[:], in_=idx1_in[k])
                ix2 = ixp.tile([P, CAP // 16], i16, tag="ix2")
                nc.sync.dma_start(out=ix2[:], in_=idx2_in[k])
                g1 = gp.tile([P, C, 2 * D], f32, tag="g1")
                nc.gpsimd.dma_gather(g1[:], KV1[:], ix1[:], CAP, CAP, 2 * D,
                                     queue_num=0, single_packet=False)
                g2 = gp.tile([P, C, 2 * D], f32, tag="g2")
                nc.gpsimd.dma_gather(g2[:], KV2[:], ix2[:], CAP, CAP, 2 * D,
                                     queue_num=1, single_packet=False)
                m1 = mp.tile([P, C, D + 1], f32, tag="m1")
                m2 = mp.tile([P, C, D + 1], f32, tag="m2")
                nc.vector.tensor_tensor(out=m1[:, :, 0:D], in0=g1[:, :, 0:D],
                                        in1=g2[:, :, 0:D], op=mult)
                tt = smp.tile([P, C], f32, tag="tt")
                nc.vector.reduce_sum(out=tt[:], in_=m1[:, :, 0:D], axis=AX)
                et = smp.tile([P, C], f32, tag="et")
                nc.scalar.activation(out=et[:], in_=tt[:], func=Exp, scale=0.125)
                nc.sync.dma_start(out=e_buf[k], in_=et[:])
                ebc = et[:].rearrange("p (c o) -> p c o", o=1).to_broadcast([P, C, D])
                nc.vector.tensor_tensor(out=m1[:, :, 0:D], in0=ebc,
                                        in1=g2[:, :, D:2 * D], op=mult)
                nc.vector.tensor_copy(out=m1[:, :, D], in_=et[:])
                nc.vector.tensor_tensor(out=m2[:, :, 0:D], in0=ebc,
                                        in1=g1[:, :, D:2 * D], op=mult)
                nc.vector.tensor_copy(out=m2[:, :, D], in_=et[:])
                nc.gpsimd.dma_scatter_add(U1[:, 0:D + 1], m1[:], ix1[:], CAP, CAP,
                                          D + 1, elem_step=2 * D,
                                          queue_num=2, single_packet=False)
                nc.gpsimd.dma_scatter_add(U2[:, 0:D + 1], m2[:], ix2[:], CAP, CAP,
                                          D + 1, elem_step=2 * D,
                                          queue_num=3, single_packet=False)

            # ---- reduce partials across cores ----
            nc.gpsimd.collective_compute(
                "ReduceScatter", add, replica_groups=groups,
                ins=[U1[0:NODE_PAD, :]], outs=[u1rs[:]])
            nc.gpsimd.collective_compute(
                "ReduceScatter", add, replica_groups=groups,
                ins=[U2[0:NODE_PAD, :]], outs=[u2rs[:]])

            # ---- epilogue on owned nodes ----
            for b in range(OBLK):
                rows = slice(b * P, (b + 1) * P)
                rp_t = epp.tile([P, 2], f32, tag="rp")
                for side, (urs, out_o) in enumerate(((u1rs, out1_o), (u2rs, out2_o))):
                    ub = epp.tile([P, 2 * D], f32, tag="ub")
                    nc.sync.dma_start(out=ub[:], in_=urs[rows, :])
                    sp = epp.tile([P, 1], f32, tag="sp")
                    nc.vector.tensor_scalar(out=sp[:], in0=ub[:, D:D + 1],
                                            scalar1=1e-8, scalar2=None, op0=add)
                    rt = epp.tile([P, 1], f32, tag="rt")
                    nc.vector.reciprocal(out=rt[:], in_=sp[:])
                    nc.vector.tensor_copy(out=rp_t[:, side:side + 1], in_=rt[:])
                    msg = epp.tile([P, P], f32, tag="msg")
                    nc.vector.tensor_tensor(out=msg[:, 0:D], in0=ub[:, 0:D],
                                            in1=rt[:].to_broadcast([P, D]), op=mult)
                    mtp = psp.tile([P, P], f32, tag="ntp")
                    nc.tensor.transpose(out=mtp[:], in_=msg[:], identity=ident[:])
                    mt = epp.tile([P, P], f32, tag="mt")
                    nc.vector.tensor_copy(out=mt[:], in_=mtp[:])
                    zp = psp.tile([P, D], f32, tag="zp")
                    nc.tensor.matmul(out=zp[:], lhsT=mt[0:D, :], rhs=wo_t[:],
                                     start=True, stop=True)
                    zb = epp.tile([P, D], f32, tag="zb")
                    nc.vector.tensor_tensor(out=zb[:], in0=zp[:],
                                            in1=bo_t[:], op=add)
                    zs = epp.tile([P, D], f32, tag="zs")
                    nc.vector.tensor_scalar(out=zs[:], in0=zb[:], scalar1=0.01,
                                            scalar2=None, op0=mult)
                    yt = epp.tile([P, D], f32, tag="yt")
                    nc.vector.tensor_tensor(out=yt[:], in0=zb[:], in1=zs[:], op=amax)
                    nc.sync.dma_start(out=out_o[rows, :], in_=yt[:])
                nc.sync.dma_start(out=rp_loc[rows, :], in_=rp_t[:])

            nc.gpsimd.collective_compute(
                "AllGather", mybir.AluOpType.bypass, replica_groups=groups,
                ins=[rp_loc[:]], outs=[rp_full[:]])

            # ---- build Rb table: row n = [r1[n], r2[n], 0...] ----
            for h in range(2):
                rpt = epp.tile([P, A_HALF, 2], f32, tag="rpt")
                nc.sync.dma_start(
                    out=rpt[:].rearrange("p a d -> p (a d)"),
                    in_=rp_full[h * HALF:(h + 1) * HALF, :].rearrange(
                        "(p a) d -> p (a d)", p=P))
                rbt = epp.tile([P, A_HALF, D], f32, tag="rbt")
                nc.vector.memset(rbt[:], 0.0)
                nc.vector.tensor_copy(out=rbt[:, :, 0:2], in_=rpt[:])
                nc.sync.dma_start(
                    out=Rb[h * HALF:(h + 1) * HALF, :].rearrange(
                        "(p a) d -> p (a d)", p=P),
                    in_=rbt[:].rearrange("p a d -> p (a d)"))

            if _debug:
                for kname, (t_, shp) in dbg_outs.items():
                    tot = int(np.prod(shp))
                    src_v = t_[:].rearrange(
                        " ".join("abcd"[:len(shp)]) + " -> (" + " ".join("abcd"[:len(shp)]) + ")")
                    dst_v = dbg_decl[kname][:].rearrange(
                        " ".join("abcd"[:len(shp)]) + " -> (" + " ".join("abcd"[:len(shp)]) + ")")
                    x = tot // P
                    sv = src_v.rearrange("(p x) -> p x", p=P)
                    dv = dst_v.rearrange("(p x) -> p x", p=P)
                    for off in range(0, x, 4096):
                        w = min(4096, x - off)
                        bt = epp.tile([P, 4096], f32, tag="zero")
                        nc.sync.dma_start(out=bt[:, 0:w], in_=sv[:, off:off + w])
                        nc.sync.dma_start(out=dv[:, off:off + w], in_=bt[:, 0:w])

            # ---- pass 2: per-edge weights ----
            for k in range(NCHUNK):
                ix1 = ixp.tile([P, CAP // 16], i16, tag="ix1")
                nc.sync.dma_start(out=ix1[:], in_=idx1_in[k])
                ix2 = ixp.tile([P, CAP // 16], i16, tag="ix2")
                nc.sync.dma_start(out=ix2[:], in_=idx2_in[k])
                gr1 = gp.tile([P, C, D], f32, tag="g1")
                nc.gpsimd.dma_gather(gr1[:], Rb[:], ix1[:], CAP, CAP, D,
                                     queue_num=0, single_packet=False)
                gr2 = gp.tile([P, C, D], f32, tag="g2")
                nc.gpsimd.dma_gather(gr2[:], Rb[:], ix2[:], CAP, CAP, D,
                                     queue_num=1, single_packet=False)
                et = smp.tile([P, C], f32, tag="et")
                nc.sync.dma_start(out=et[:], in_=e_buf[k])
                a1t = smp.tile([P, C], f32, tag="a1t")
                nc.vector.tensor_tensor(out=a1t[:], in0=et[:], in1=gr1[:, :, 0],
                                        op=mult)
                nc.sync.dma_start(out=a1_o[k], in_=a1t[:])
                a2t = smp.tile([P, C], f32, tag="a2t")
                nc.vector.tensor_tensor(out=a2t[:], in0=et[:], in1=gr2[:, :, 1],
                                        op=mult)
                nc.sync.dma_start(out=a2_o[k], in_=a2t[:])

    nc.finalize()

    in_maps = []
    for c in range(NCORE):
        in_maps.append({
            "node1p": node1p, "node2p": node2p, "wkv": wkv, "wo": Wo,
            "bo": bo2, "idx1": idx1_np[c], "idx2": idx2_np[c],
        })
    results = _run_spmd(nc, in_maps, NCORE)

    class _Res:
        pass

    res = _Res()
    res.results = results

    msg1 = np.concatenate([res.results[c]["out1s"] for c in range(NCORE)])[:N1]
    msg2 = np.concatenate([res.results[c]["out2s"] for c in range(NCORE)])[:N2]
    a1 = np.zeros(E, np.float32)
    a2 = np.zeros(E, np.float32)
    for c in range(NCORE):
        ra1 = res.results[c]["a1s"]  # [NCHUNK, P, C]
        ra2 = res.results[c]["a2s"]
        for k in range(NCHUNK):
            b = members[c][k]
            if b.size == 0:
                continue
            j = np.arange(b.size)
            a1[b] = ra1[k, j % P, j // P]
            a2[b] = ra2[k, j % P, j // P]
    if _debug:
        return (msg1, msg2, a1[:, None], a2[:, None]), res, members, core_bins
    return (msg1, msg2, a1[:, None], a2[:, None])
